# revision 12
# baseline (speedup 1.0000x reference)
"""Cascaded 5-level stride-2 spatial downsample on Trainium2 (8 NeuronCores).

Math (from the degenerate depthwise convs): down_k = x[:, :, ::2**k, ::2**k]
for k = 1..5 on x of shape (4, 3, 4096, 4096) f32.

Sharding: pure data parallel over H. Core m owns rows [512m, 512(m+1)) of
every (batch, channel) image; 512 is divisible by 32 so every output level
shards cleanly along H. Each core receives its slab flattened to
(12*512, 4096); outputs are concatenated along H on the host.

Device strategy (memory-bound; HBM traffic is the floor: 50.3MB read of the
even source rows + 33.5MB of outputs per core):
  - One pass, 12 tile iterations (one per image). DMA in only the EVEN rows
    (16KB contiguous chunks, stride 32KB) on the SP HWDGE ring.
  - Column subsampling on-chip with strided-AP DVE copies (d1, d2, then a
    chain c3 -> c4 -> c5 halving columns each step).
  - Row subsampling for down3/4/5 happens inside the output DMA: the write
    reads every 2nd/4th/8th SBUF partition (partition-strided source AP) and
    stores contiguously to DRAM. Writes go on the ACT HWDGE ring so reads
    and writes pipeline independently.
"""

import numpy as np

import concourse.bacc as bacc
import concourse.bass as bass
import concourse.mybir as mybir
import concourse.tile as tile
from concourse.bass_utils import run_bass_kernel_spmd

NCORES = 8
NIMG = 12          # 4 batch * 3 channels
H, W = 4096, 4096
SLAB = H // NCORES  # 512 rows per core per image
FLATROWS = NIMG * SLAB  # 6144

F32 = mybir.dt.float32


def build_nc():
    nc = bacc.Bacc()
    x = nc.dram_tensor("x", (FLATROWS, W), F32, kind="ExternalInput")
    d = {
        k: nc.dram_tensor(
            f"d{k}", (FLATROWS >> k, W >> k), F32, kind="ExternalOutput"
        )
        for k in range(1, 6)
    }

    with tile.TileContext(nc) as tc:
        with tc.tile_pool(name="io", bufs=3) as pool:
            # iteration t covers flat source rows [512t, 512t+512); partition p
            # holds source rows 512t + 4p + 2j (j in {0,1}), i.e. down1 rows
            # 256t + 2p + j and down2 row 128t + p.
            #
            # Every DMA is SP-issued so they all share one HWDGE ring: the
            # SDMA engines then drain them strictly in issue order, and HBM
            # sees multi-MB read bursts alternating with write bursts instead
            # of packet-level read/write mixing (bus-turnaround churn).  To
            # avoid head-of-line blocking at the SP sequencer (a write waits
            # on DVE), iteration t's writes are emitted after iteration
            # t+2's read (two-iteration lag doubles the burst size).
            pend = []  # deferred write args, emitted with a 2-iteration lag
            LAG = 2

            def emit_writes(t, d1v, d2t, c3, c4, c5):
                dst1 = d[1][256 * t : 256 * (t + 1), :].rearrange(
                    "(p j) w -> p j w", j=2
                )
                nc.sync.dma_start(out=dst1, in_=d1v)
                nc.sync.dma_start(
                    out=d[2][128 * t : 128 * (t + 1), :], in_=d2t[:]
                )
                # row subsample via partition-strided SBUF reads in the DMA
                nc.sync.dma_start(
                    out=d[3][64 * t : 64 * t + 64, :], in_=c3[:][::2, :]
                )
                nc.sync.dma_start(
                    out=d[4][32 * t : 32 * t + 32, :], in_=c4[:][::4, :]
                )
                nc.sync.dma_start(
                    out=d[5][16 * t : 16 * t + 16, :], in_=c5[:][::8, :]
                )

            for t in range(FLATROWS // 512):
                src = x[512 * t : 512 * (t + 1) : 2, :]  # (256, 4096) even rows
                src3 = src.rearrange("(p j) w -> p j w", j=2)
                xin = pool.tile([128, 2 * W], F32, tag="xin")
                xin3 = xin[:].rearrange("p (j w) -> p j w", j=2)
                nc.sync.dma_start(out=xin3, in_=src3)

                if len(pend) >= LAG:
                    emit_writes(*pend.pop(0))

                d1t = pool.tile([128, W], F32, tag="d1")
                d1v = d1t[:].rearrange("p (j w) -> p j w", j=2)
                nc.vector.tensor_copy(out=d1v, in_=xin3[:, :, ::2])

                d2t = pool.tile([128, W // 4], F32, tag="d2")
                nc.vector.tensor_copy(out=d2t[:], in_=xin3[:, 0, ::4])

                # column chain for the deeper levels; partition p still maps
                # to down2 row 128t + p.
                c3 = pool.tile([128, W // 8], F32, tag="c3")
                nc.vector.tensor_copy(out=c3[:], in_=d2t[:, ::2])
                c4 = pool.tile([128, W // 16], F32, tag="c4")
                nc.vector.tensor_copy(out=c4[:], in_=c3[:, ::2])
                c5 = pool.tile([128, W // 32], F32, tag="c5")
                nc.vector.tensor_copy(out=c5[:], in_=c4[:, ::2])

                pend.append((t, d1v, d2t, c3, c4, c5))

            for args in pend:
                emit_writes(*args)
    nc.finalize()
    return nc


_NC_CACHE = None


def _get_nc():
    global _NC_CACHE
    if _NC_CACHE is None:
        _NC_CACHE = build_nc()
    return _NC_CACHE


def run(x, trace=False):
    """x: full (4, 3, 4096, 4096) f32. Returns (results, tuple_of_5_outputs)."""
    xr = np.asarray(x, dtype=np.float32).reshape(NIMG, H, W)
    in_maps = [
        {
            "x": np.ascontiguousarray(
                xr[:, SLAB * m : SLAB * (m + 1), :]
            ).reshape(FLATROWS, W)
        }
        for m in range(NCORES)
    ]
    nc = _get_nc()
    res = run_bass_kernel_spmd(nc, in_maps, list(range(NCORES)), trace=trace)
    outs = []
    for k in range(1, 6):
        shards = [
            res.results[m][f"d{k}"].reshape(4, 3, SLAB >> k, W >> k)
            for m in range(NCORES)
        ]
        outs.append(np.concatenate(shards, axis=2))
    return res, tuple(outs)


def kernel(x):
    _, outs = run(x)
    return outs


# revision 13
# speedup vs baseline: 1.0188x; 1.0188x over previous
"""Cascaded 5-level stride-2 spatial downsample on Trainium2 (8 NeuronCores).

Math (from the degenerate depthwise convs): down_k = x[:, :, ::2**k, ::2**k]
for k = 1..5 on x of shape (4, 3, 4096, 4096) f32.

Sharding: pure data parallel over H. Core m owns rows [512m, 512(m+1)) of
every (batch, channel) image; 512 is divisible by 32 so every output level
shards cleanly along H. Each core receives its slab flattened to
(12*512, 4096); outputs are concatenated along H on the host.

Device strategy (memory-bound; HBM traffic is the floor: 50.3MB read of the
even source rows + 33.5MB of outputs per core):
  - One pass, 12 tile iterations (one per image). DMA in only the EVEN rows
    (16KB contiguous chunks, stride 32KB) on the SP HWDGE ring.
  - Column subsampling on-chip with strided-AP DVE copies (d1, d2, then a
    chain c3 -> c4 -> c5 halving columns each step).
  - Row subsampling for down3/4/5 happens inside the output DMA: the write
    reads every 2nd/4th/8th SBUF partition (partition-strided source AP) and
    stores contiguously to DRAM. Writes go on the ACT HWDGE ring so reads
    and writes pipeline independently.
"""

import numpy as np

import concourse.bacc as bacc
import concourse.bass as bass
import concourse.mybir as mybir
import concourse.tile as tile
from concourse.bass_utils import run_bass_kernel_spmd

NCORES = 8
NIMG = 12          # 4 batch * 3 channels
H, W = 4096, 4096
SLAB = H // NCORES  # 512 rows per core per image
FLATROWS = NIMG * SLAB  # 6144

F32 = mybir.dt.float32


def build_nc():
    nc = bacc.Bacc()
    x = nc.dram_tensor("x", (FLATROWS, W), F32, kind="ExternalInput")
    d = {
        k: nc.dram_tensor(
            f"d{k}", (FLATROWS >> k, W >> k), F32, kind="ExternalOutput"
        )
        for k in range(1, 6)
    }

    with tile.TileContext(nc) as tc:
        with tc.tile_pool(name="io", bufs=3) as pool:
            # iteration t covers flat source rows [512t, 512t+512); partition p
            # holds source rows 512t + 4p + 2j (j in {0,1}), i.e. down1 rows
            # 256t + 2p + j and down2 row 128t + p.
            #
            # Every DMA is SP-issued so they all share one HWDGE ring: the
            # SDMA engines then drain them strictly in issue order, and HBM
            # sees multi-MB read bursts alternating with write bursts instead
            # of packet-level read/write mixing (bus-turnaround churn).  To
            # avoid head-of-line blocking at the SP sequencer (a write waits
            # on DVE), iteration t's writes are emitted after iteration
            # t+1's read.
            pend = []  # deferred write args, emitted with a 1-iteration lag
            LAG = 1

            def emit_writes(t, d1v, d2t, c3, c4, c5):
                dst1 = d[1][256 * t : 256 * (t + 1), :].rearrange(
                    "(p j) w -> p j w", j=2
                )
                nc.sync.dma_start(out=dst1, in_=d1v)
                nc.sync.dma_start(
                    out=d[2][128 * t : 128 * (t + 1), :], in_=d2t[:]
                )
                # row subsample via partition-strided SBUF reads in the DMA
                nc.sync.dma_start(
                    out=d[3][64 * t : 64 * t + 64, :], in_=c3[:][::2, :]
                )
                nc.sync.dma_start(
                    out=d[4][32 * t : 32 * t + 32, :], in_=c4[:][::4, :]
                )
                nc.sync.dma_start(
                    out=d[5][16 * t : 16 * t + 16, :], in_=c5[:][::8, :]
                )

            for t in range(FLATROWS // 512):
                src = x[512 * t : 512 * (t + 1) : 2, :]  # (256, 4096) even rows
                src3 = src.rearrange("(p j) w -> p j w", j=2)
                xin = pool.tile([128, 2 * W], F32, tag="xin")
                xin3 = xin[:].rearrange("p (j w) -> p j w", j=2)
                nc.sync.dma_start(out=xin3, in_=src3)

                if len(pend) >= LAG:
                    emit_writes(*pend.pop(0))

                d1t = pool.tile([128, W], F32, tag="d1")
                d1v = d1t[:].rearrange("p (j w) -> p j w", j=2)
                nc.vector.tensor_copy(out=d1v, in_=xin3[:, :, ::2])

                d2t = pool.tile([128, W // 4], F32, tag="d2")
                nc.vector.tensor_copy(out=d2t[:], in_=xin3[:, 0, ::4])

                # column chain for the deeper levels; partition p still maps
                # to down2 row 128t + p.
                c3 = pool.tile([128, W // 8], F32, tag="c3")
                nc.vector.tensor_copy(out=c3[:], in_=d2t[:, ::2])
                c4 = pool.tile([128, W // 16], F32, tag="c4")
                nc.vector.tensor_copy(out=c4[:], in_=c3[:, ::2])
                c5 = pool.tile([128, W // 32], F32, tag="c5")
                nc.vector.tensor_copy(out=c5[:], in_=c4[:, ::2])

                pend.append((t, d1v, d2t, c3, c4, c5))

            for args in pend:
                emit_writes(*args)
    nc.finalize()
    return nc


_NC_CACHE = None


def _get_nc():
    global _NC_CACHE
    if _NC_CACHE is None:
        _NC_CACHE = build_nc()
    return _NC_CACHE


def run(x, trace=False):
    """x: full (4, 3, 4096, 4096) f32. Returns (results, tuple_of_5_outputs)."""
    xr = np.asarray(x, dtype=np.float32).reshape(NIMG, H, W)
    in_maps = [
        {
            "x": np.ascontiguousarray(
                xr[:, SLAB * m : SLAB * (m + 1), :]
            ).reshape(FLATROWS, W)
        }
        for m in range(NCORES)
    ]
    nc = _get_nc()
    res = run_bass_kernel_spmd(nc, in_maps, list(range(NCORES)), trace=trace)
    outs = []
    for k in range(1, 6):
        shards = [
            res.results[m][f"d{k}"].reshape(4, 3, SLAB >> k, W >> k)
            for m in range(NCORES)
        ]
        outs.append(np.concatenate(shards, axis=2))
    return res, tuple(outs)


def kernel(x):
    _, outs = run(x)
    return outs


# revision 14
# speedup vs baseline: 1.0270x; 1.0081x over previous
"""Cascaded 5-level stride-2 spatial downsample on Trainium2 (8 NeuronCores).

Math (from the degenerate depthwise convs): down_k = x[:, :, ::2**k, ::2**k]
for k = 1..5 on x of shape (4, 3, 4096, 4096) f32.

Sharding: pure data parallel over H. Core m owns rows [512m, 512(m+1)) of
every (batch, channel) image; 512 is divisible by 32 so every output level
shards cleanly along H. Each core receives its slab flattened to
(12*512, 4096); outputs are concatenated along H on the host.

Device strategy (memory-bound; HBM traffic is the floor: 50.3MB read of the
even source rows + 33.5MB of outputs per core):
  - One pass, 12 tile iterations (one per image). DMA in only the EVEN rows
    (16KB contiguous chunks, stride 32KB) on the SP HWDGE ring.
  - Column subsampling on-chip with strided-AP DVE copies (d1, d2, then a
    chain c3 -> c4 -> c5 halving columns each step).
  - Row subsampling for down3/4/5 happens inside the output DMA: the write
    reads every 2nd/4th/8th SBUF partition (partition-strided source AP) and
    stores contiguously to DRAM.
  - All DMAs share the SP HWDGE ring (strict FIFO) so HBM sees multi-MB
    read bursts alternating with write bursts instead of packet-level
    read/write mixing; writes are emitted one iteration late to keep the
    SP sequencer from head-of-line blocking on compute.
"""

import numpy as np

import concourse.bacc as bacc
import concourse.mybir as mybir
import concourse.tile as tile
from concourse.bass_utils import run_bass_kernel_spmd

NCORES = 8
NIMG = 12          # 4 batch * 3 channels
H, W = 4096, 4096
SLAB = H // NCORES  # 512 rows per core per image
FLATROWS = NIMG * SLAB  # 6144

F32 = mybir.dt.float32


def build_nc():
    nc = bacc.Bacc()
    x = nc.dram_tensor("x", (FLATROWS, W), F32, kind="ExternalInput")
    d = {
        k: nc.dram_tensor(
            f"d{k}", (FLATROWS >> k, W >> k), F32, kind="ExternalOutput"
        )
        for k in range(1, 6)
    }

    with tile.TileContext(nc) as tc:
        with tc.tile_pool(name="io", bufs=3) as pool:
            # iteration t covers flat source rows [512t, 512t+512); partition p
            # holds source rows 512t + 4p + 2j (j in {0,1}), i.e. down1 rows
            # 256t + 2p + j and down2 row 128t + p.
            #
            # Every DMA is SP-issued so they all share one HWDGE ring: the
            # SDMA engines then drain them strictly in issue order, and HBM
            # sees multi-MB read bursts alternating with write bursts instead
            # of packet-level read/write mixing (bus-turnaround churn).  To
            # avoid head-of-line blocking at the SP sequencer (a write waits
            # on DVE), iteration t's writes are emitted after iteration
            # t+1's read.
            pend = []  # deferred write args, emitted with a 1-iteration lag
            LAG = 1

            def emit_writes(t, d1v, d2t, c3, c4, c5):
                dst1 = d[1][256 * t : 256 * (t + 1), :].rearrange(
                    "(p j) w -> p j w", j=2
                )
                nc.sync.dma_start(out=dst1, in_=d1v)
                nc.sync.dma_start(
                    out=d[2][128 * t : 128 * (t + 1), :], in_=d2t[:]
                )
                # row subsample via partition-strided SBUF reads in the DMA
                nc.sync.dma_start(
                    out=d[3][64 * t : 64 * t + 64, :], in_=c3[:][::2, :]
                )
                nc.sync.dma_start(
                    out=d[4][32 * t : 32 * t + 32, :], in_=c4[:][::4, :]
                )
                nc.sync.dma_start(
                    out=d[5][16 * t : 16 * t + 16, :], in_=c5[:][::8, :]
                )

            for t in range(FLATROWS // 512):
                src = x[512 * t : 512 * (t + 1) : 2, :]  # (256, 4096) even rows
                src3 = src.rearrange("(p j) w -> p j w", j=2)
                xin = pool.tile([128, 2 * W], F32, tag="xin")
                xin3 = xin[:].rearrange("p (j w) -> p j w", j=2)
                nc.sync.dma_start(out=xin3, in_=src3)

                if len(pend) >= LAG:
                    emit_writes(*pend.pop(0))

                d1t = pool.tile([128, W], F32, tag="d1")
                d1v = d1t[:].rearrange("p (j w) -> p j w", j=2)
                nc.vector.tensor_copy(out=d1v, in_=xin3[:, :, ::2])

                d2t = pool.tile([128, W // 4], F32, tag="d2")
                nc.vector.tensor_copy(out=d2t[:], in_=xin3[:, 0, ::4])

                # column chain for the deeper levels; partition p still maps
                # to down2 row 128t + p.
                c3 = pool.tile([128, W // 8], F32, tag="c3")
                nc.vector.tensor_copy(out=c3[:], in_=d2t[:, ::2])
                c4 = pool.tile([128, W // 16], F32, tag="c4")
                nc.vector.tensor_copy(out=c4[:], in_=c3[:, ::2])
                c5 = pool.tile([128, W // 32], F32, tag="c5")
                nc.vector.tensor_copy(out=c5[:], in_=c4[:, ::2])

                pend.append((t, d1v, d2t, c3, c4, c5))

            for args in pend:
                emit_writes(*args)
    nc.finalize()
    return nc


_NC_CACHE = None


def _get_nc():
    global _NC_CACHE
    if _NC_CACHE is None:
        _NC_CACHE = build_nc()
    return _NC_CACHE


def run(x, trace=False):
    """x: full (4, 3, 4096, 4096) f32. Returns (results, tuple_of_5_outputs)."""
    xr = np.asarray(x, dtype=np.float32).reshape(NIMG, H, W)
    in_maps = [
        {
            "x": np.ascontiguousarray(
                xr[:, SLAB * m : SLAB * (m + 1), :]
            ).reshape(FLATROWS, W)
        }
        for m in range(NCORES)
    ]
    nc = _get_nc()
    res = run_bass_kernel_spmd(nc, in_maps, list(range(NCORES)), trace=trace)
    outs = []
    for k in range(1, 6):
        shards = [
            res.results[m][f"d{k}"].reshape(4, 3, SLAB >> k, W >> k)
            for m in range(NCORES)
        ]
        outs.append(np.concatenate(shards, axis=2))
    return res, tuple(outs)


def kernel(x):
    _, outs = run(x)
    return outs


# revision 17
# speedup vs baseline: 1.2289x; 1.1966x over previous
"""Cascaded 5-level stride-2 spatial downsample on Trainium2 (8 NeuronCores).

Math (from the degenerate depthwise convs): down_k = x[:, :, ::2**k, ::2**k]
for k = 1..5 on x of shape (4, 3, 4096, 4096) f32.

Sharding: pure data parallel over H. Core m owns rows [512m, 512(m+1)) of
every (batch, channel) image; 512 is divisible by 32 so every output level
shards cleanly along H. Each core receives its slab flattened to
(12*512, 4096); outputs are concatenated along H on the host.

Device strategy (memory-bound; HBM traffic is the floor: 50.3MB read of the
even source rows + 33.5MB of outputs per core):
  - One pass, 12 tile iterations (one per image). DMA in only the EVEN rows
    (16KB contiguous chunks, stride 32KB) on the SP HWDGE ring.
  - Column subsampling on-chip with strided-AP DVE copies (d1, d2, then a
    chain c3 -> c4 -> c5 halving columns each step).
  - Row subsampling for down3/4/5 happens inside the output DMA: the write
    reads every 2nd/4th/8th SBUF partition (partition-strided source AP) and
    stores contiguously to DRAM.
  - All DMAs share the SP HWDGE ring (strict FIFO) so HBM sees multi-MB
    read bursts alternating with write bursts instead of packet-level
    read/write mixing; writes are emitted one iteration late to keep the
    SP sequencer from head-of-line blocking on compute.
"""

import numpy as np

import concourse.bacc as bacc
import concourse.mybir as mybir
import concourse.tile as tile
from concourse.bass_utils import run_bass_kernel_spmd

NCORES = 8
NIMG = 12          # 4 batch * 3 channels
H, W = 4096, 4096
SLAB = H // NCORES  # 512 rows per core per image
FLATROWS = NIMG * SLAB  # 6144

F32 = mybir.dt.float32


def build_nc():
    nc = bacc.Bacc()
    x = nc.dram_tensor("x", (FLATROWS, W), F32, kind="ExternalInput")
    d = {
        k: nc.dram_tensor(
            f"d{k}", (FLATROWS >> k, W >> k), F32, kind="ExternalOutput"
        )
        for k in range(1, 6)
    }

    with tile.TileContext(nc) as tc:
        with (
            tc.tile_pool(name="io", bufs=3) as pool,
            tc.tile_pool(name="wr", bufs=4) as wpool,
        ):
            # iteration t covers flat source rows [512t, 512t+512); partition p
            # holds source rows 512t + 4p + 2j (j in {0,1}), i.e. down1 rows
            # 256t + 2p + j and down2 row 128t + p.
            #
            # Every DMA is SP-issued so they all share one HWDGE ring: the
            # SDMA engines then drain them strictly in issue order, and HBM
            # sees multi-MB read bursts alternating with write bursts instead
            # of packet-level read/write mixing (bus-turnaround churn).  To
            # avoid head-of-line blocking at the SP sequencer (a write waits
            # on DVE), iteration t's writes are emitted after iteration
            # t+1's read.
            pend = []  # deferred write args, emitted with a 2-iteration lag
            LAG = 2

            def emit_writes(t, d1v, d2t, c3, c4, c5):
                dst1 = d[1][256 * t : 256 * (t + 1), :].rearrange(
                    "(p j) w -> p j w", j=2
                )
                nc.sync.dma_start(out=dst1, in_=d1v)
                nc.sync.dma_start(
                    out=d[2][128 * t : 128 * (t + 1), :], in_=d2t[:]
                )
                # row subsample via partition-strided SBUF reads in the DMA
                nc.sync.dma_start(
                    out=d[3][64 * t : 64 * t + 64, :], in_=c3[:][::2, :]
                )
                nc.sync.dma_start(
                    out=d[4][32 * t : 32 * t + 32, :], in_=c4[:][::4, :]
                )
                nc.sync.dma_start(
                    out=d[5][16 * t : 16 * t + 16, :], in_=c5[:][::8, :]
                )

            for t in range(FLATROWS // 512):
                src = x[512 * t : 512 * (t + 1) : 2, :]  # (256, 4096) even rows
                src3 = src.rearrange("(p j) w -> p j w", j=2)
                xin = pool.tile([128, 2 * W], F32, tag="xin")
                xin3 = xin[:].rearrange("p (j w) -> p j w", j=2)
                nc.sync.dma_start(out=xin3, in_=src3)

                if len(pend) >= LAG:
                    emit_writes(*pend.pop(0))

                d1t = wpool.tile([128, W], F32, tag="d1")
                d1v = d1t[:].rearrange("p (j w) -> p j w", j=2)
                nc.vector.tensor_copy(out=d1v, in_=xin3[:, :, ::2])

                d2t = wpool.tile([128, W // 4], F32, tag="d2")
                nc.vector.tensor_copy(out=d2t[:], in_=xin3[:, 0, ::4])

                # column chain for the deeper levels; partition p still maps
                # to down2 row 128t + p.
                c3 = wpool.tile([128, W // 8], F32, tag="c3")
                nc.vector.tensor_copy(out=c3[:], in_=d2t[:, ::2])
                c4 = wpool.tile([128, W // 16], F32, tag="c4")
                nc.vector.tensor_copy(out=c4[:], in_=c3[:, ::2])
                c5 = wpool.tile([128, W // 32], F32, tag="c5")
                nc.vector.tensor_copy(out=c5[:], in_=c4[:, ::2])

                pend.append((t, d1v, d2t, c3, c4, c5))

            for args in pend:
                emit_writes(*args)
    nc.finalize()
    return nc


_NC_CACHE = None


def _get_nc():
    global _NC_CACHE
    if _NC_CACHE is None:
        _NC_CACHE = build_nc()
    return _NC_CACHE


def run(x, trace=False):
    """x: full (4, 3, 4096, 4096) f32. Returns (results, tuple_of_5_outputs)."""
    xr = np.asarray(x, dtype=np.float32).reshape(NIMG, H, W)
    in_maps = [
        {
            "x": np.ascontiguousarray(
                xr[:, SLAB * m : SLAB * (m + 1), :]
            ).reshape(FLATROWS, W)
        }
        for m in range(NCORES)
    ]
    nc = _get_nc()
    res = run_bass_kernel_spmd(nc, in_maps, list(range(NCORES)), trace=trace)
    outs = []
    for k in range(1, 6):
        shards = [
            res.results[m][f"d{k}"].reshape(4, 3, SLAB >> k, W >> k)
            for m in range(NCORES)
        ]
        outs.append(np.concatenate(shards, axis=2))
    return res, tuple(outs)


def kernel(x):
    _, outs = run(x)
    return outs
